# revision 24
# baseline (speedup 1.0000x reference)
"""Trainium2 Bass kernel for a dense transformer block (pre-LN, MHA + MLP).

Full inputs in, full outputs out. Sharding: 8 cores = (batch, seq-half).
Each core computes K/V over its batch element's full 1024 tokens and
Q/attention/MLP over its own 512 tokens (host permutes tokens so the core's
own half is always rows 0..511 — softmax over keys is permutation invariant).
No collectives needed.

Precision strategy:
  - Attention-side GEMMs (K, V, Q, attn@V, proj) run as fp8e4m3 DoubleRow
    matmuls.  Weights are prescaled x16 on the host to sit in e4m3's normal
    range; the 1/16 (or 1/256) correction folds into the psum drain.
  - fc1 stays bf16 (two fp8 MLP layers would blow the 2e-2 error gate);
    fc2 runs fp8 DoubleRow (acts + weights), which measures ~1.7e-2.
  - Scores (64-deep contraction) stay bf16; DoubleRow needs 128-pairs.

Schedule (the big wins over the first working version):
  - V sweep is interleaved into the LN1 loop: V(token tile t) only needs
    tile t's hT columns, so it streams right behind the LN1 transposes
    instead of waiting for all of LN1.
  - The per-head attention loop is software-pipelined by one head:
    attn@V(h-1) is emitted AFTER scores/exp(h), so the in-order PE queue
    never stalls waiting for the Scalar engine's exp — the phase runs at
    the exp roofline (~60us) instead of ping-ponging.
  - fc2 weight DMAs (both halves) are spread through the fc1 loop so fc2
    never waits on HBM; fc1 slabs are split into 2 chunks / 2 rings.
  - x2 is rewritten in place to (x + attn_resid + pb + f2b) during fc1 so
    the fc2 drain is scale + one add.
"""

import sys

sys.path.insert(0, "/opt/trn_rl_repo")

import numpy as np

import concourse.bass as bass
import concourse.bacc as bacc
import concourse.mybir as mybir
import concourse.tile as tile
from concourse.bass_utils import run_bass_kernel_spmd
from concourse.masks import make_identity

P = 128
C = 1024
HEADS = 16
DH = 64
HID = 4096
NTOK = 1024  # tokens per batch element (kv length)
NOWN = 512  # tokens owned by this core (q length)
SCALE = DH ** -0.5
EPS = 1e-5
WS = 16.0  # fp8 weight prescale
RWS = 1.0 / WS

F32 = mybir.dt.float32
BF16 = mybir.dt.bfloat16
FP8 = mybir.dt.float8e4
AF = mybir.ActivationFunctionType
OP = mybir.AluOpType
DR = mybir.MatmulPerfMode.DoubleRow

CT = C // P  # 8 column tiles of the model dim
TT = NTOK // P  # 8 token tiles (kv)
QT = NOWN // P  # 4 token tiles (own)
HT = HID // P  # 32 hidden tiles
HW8 = 80  # per-head column stride in the fused vh tile (64 dims +
# ones col + pad).  The DoubleRow pair dim is the k-tile dim with stride
# HEADS*HW8 = 1280 bytes (multiple of 16 as dual-fp8 ldweights requires).


def build_program():
    nc = bacc.Bacc("TRN2", target_bir_lowering=False)

    # All big operands are pre-tiled on the host into the exact SBUF
    # destination layout, so every dma_start below is ONE contiguous
    # block copy (strided-DRAM reads measured only ~10 GB/s per ring).
    io = {}
    io["x"] = nc.dram_tensor("x", (TT, 4, P, C // 4), F32, kind="ExternalInput")
    io["qw"] = nc.dram_tensor("qw", (CT, P, CT, P), FP8, kind="ExternalInput")
    io["kw"] = nc.dram_tensor("kw", (CT, P, CT, P), FP8, kind="ExternalInput")
    io["vw"] = nc.dram_tensor(
        "vw", (2, 4, P, 2, NOWN), FP8, kind="ExternalInput"
    )
    io["pw"] = nc.dram_tensor(
        "pw", (2, 4, P, 2, NOWN), FP8, kind="ExternalInput"
    )
    io["f1w"] = nc.dram_tensor(
        "f1w", (HT, 2, P, 4, P), BF16, kind="ExternalInput"
    )
    io["f2w"] = nc.dram_tensor(
        "f2w", (2, 8, P, 4, NOWN), FP8, kind="ExternalInput"
    )
    # biases pre-transposed on host into [128, n] per-partition layout
    io["qbt"] = nc.dram_tensor("qbt", (P, CT), F32, kind="ExternalInput")
    io["kbt"] = nc.dram_tensor("kbt", (P, CT), F32, kind="ExternalInput")
    io["f1bt"] = nc.dram_tensor("f1bt", (P, HT), F32, kind="ExternalInput")
    io["vb"] = nc.dram_tensor("vb", (C,), F32, kind="ExternalInput")
    io["pb"] = nc.dram_tensor("pb", (C,), F32, kind="ExternalInput")
    io["f2b"] = nc.dram_tensor("f2b", (C,), F32, kind="ExternalInput")
    io["out"] = nc.dram_tensor(
        "out", (QT, 2, 4, P, P), F32, kind="ExternalOutput"
    )

    with tile.TileContext(nc) as tc:
        _emit(nc, tc, io)
    nc.compile()
    return nc


def _emit(nc, tc, io):
    x_d, out_d = io["x"], io["out"]

    with (
        tc.tile_pool(name="consts", bufs=1) as consts,
        tc.tile_pool(name="persist", bufs=1) as persist,
    ):

        def copy_alt(i, out, in_):
            nc.any.tensor_copy(out=out, in_=in_)

        import contextlib
        _xwa_stack = contextlib.ExitStack()
        with tc.tile_pool(name="xwb", bufs=1) as xwb:
            xwa = _xwa_stack.enter_context(tc.tile_pool(name="xwa", bufs=1))

            # DMA issue costs ~650ns on an engine queue; spread issues over
            # three queues (all idle at startup) so enqueue isn't serial
            engs = [nc.sync, nc.gpsimd, nc.scalar]
            _ei = [0]

            def dma(out, in_):
                engs[_ei[0] % 3].dma_start(out, in_)
                _ei[0] += 1

            # ---- x tiles first: LN1 is the critical path at startup ----
            xt_all = []
            for t in range(TT):
                pool_t = xwb if t < QT else xwa
                xo = pool_t.tile([P, C], F32, tag=f"xo{t}", name=f"xo{t}")
                for q in range(4):
                    cs = slice(q * 256, (q + 1) * 256)
                    dma(xo[:, cs], x_d[t, q, :, :])
                xt_all.append(xo)
            x_own = xt_all[:QT]

            # vb: only a single-partition row is ever needed (rank-1 trick)
            vbrow = xwa.tile([1, C], F32, tag="vbrow", name="vbrow")
            nc.sync.dma_start(
                vbrow, bass.AP(tensor=io["vb"], offset=0, ap=[[0, 1], [1, C]])
            )
            nc.vector.tensor_scalar_mul(vbrow, vbrow, WS)
            vb8 = xwa.tile([1, C], FP8, tag="vb8", name="vb8")
            nc.vector.tensor_copy(out=vb8, in_=vbrow)
            ones8 = xwa.tile([1, P], FP8, tag="ones8", name="ones8")
            nc.vector.memset(ones8, 1.0)

            # fp8 V weight slabs right behind the early x tiles (V streams
            # inside the LN1 loop)
            vslab = [
                xwa.tile([P, CT, NOWN], FP8, tag=f"vs{ns}", name=f"vs{ns}")
                for ns in range(2)
            ]
            for ns in range(2):
                for j in range(4):
                    dma(
                        vslab[ns][:, 2 * j : 2 * j + 2, :],
                        io["vw"][ns, j, :, :, :],
                    )

            # ---------- constants (bias DMAs are small; issue before the
            # big k/q/proj weight slabs so the ft-loop isn't bias-gated) --
            ident = consts.tile([P, P], BF16)
            with tc.tile_pool(name="ident_tmp", bufs=1) as ident_tmp:
                ident_f32 = ident_tmp.tile([P, P], F32)
                make_identity(nc, ident_f32)
                nc.vector.tensor_copy(out=ident, in_=ident_f32)
            eps_tile = consts.tile([P, 1], F32)
            nc.vector.memset(eps_tile, EPS)
            eps256 = consts.tile([P, 1], F32)
            nc.vector.memset(eps256, EPS * (WS * WS) ** 2)

            qbT = consts.tile([P, CT], F32)
            nc.sync.dma_start(qbT, io["qbt"][:, :])
            kbT = consts.tile([P, CT], F32)
            nc.sync.dma_start(kbT, io["kbt"][:, :])
            f1bT = consts.tile([P, HT], F32)
            nc.sync.dma_start(f1bT, io["f1bt"][:, :])

            kslab = [
                xwa.tile([P, CT, P], FP8, tag=f"ks{ft}", name=f"ks{ft}")
                for ft in range(CT)
            ]
            qslab = [
                xwa.tile([P, CT, P], FP8, tag=f"qs{ft}", name=f"qs{ft}")
                for ft in range(CT)
            ]
            for ft in range(CT):
                dma(kslab[ft], io["kw"][ft, :, :, :])
                dma(qslab[ft], io["qw"][ft, :, :, :])
            # pslab DMAs are issued inside the ft-loop (proj runs late)
            pslab = [
                xwb.tile([P, CT, NOWN], FP8, tag=f"pws{ns}", name=f"pws{ns}")
                for ns in range(2)
            ]

            def bcast_const(src_d, n):
                # split across 4 DMA rings; a single 512KB broadcast DMA
                # would occupy one ring for ~30us
                t = consts.tile([P, n], F32)
                for q in range(4):
                    nq = n // 4
                    src = bass.AP(
                        tensor=src_d, offset=q * nq, ap=[[0, P], [1, nq]]
                    )
                    nc.sync.dma_start(t[:, q * nq : (q + 1) * nq], src)
                return t

            # x2 split into (tq, ns) halves so LN2 stats can start after the
            # first proj drain of each token tile (deps are tile-granular)
            x2 = {
                (t, ns): persist.tile(
                    [P, NOWN], F32, tag=f"x2_{t}_{ns}", name=f"x2_{t}_{ns}"
                )
                for t in range(QT)
                for ns in range(2)
            }

            def layernorm_tile(temps, halves, eps_t=None):
                """halves: two [128, 512] fp32 views/tiles of one token tile.
                Returns two normalized bf16 half tiles [128, 512].

                Stats on DVE, h-write on Act (Identity is in every act table).
                """
                stats = temps.tile([P, 2, 6], F32, tag="ln_stats", name="st")
                for sg in range(2):
                    nc.vector.bn_stats(out=stats[:, sg, :], in_=halves[sg])
                mv = temps.tile([P, 2], F32, tag="ln_mv", name="mv")
                nc.vector.bn_aggr(out=mv[:], in_=stats[:])
                rstd = temps.tile([P, 1], F32, tag="ln_rstd", name="rstd")
                nc.scalar.activation(
                    out=rstd, in_=mv[:, 1:2], func=AF.Sqrt,
                    bias=eps_t if eps_t is not None else eps_tile, scale=1.0,
                )
                nc.vector.reciprocal(out=rstd, in_=rstd)
                nmr = temps.tile([P, 1], F32, tag="ln_nmr", name="nmr")
                nc.vector.tensor_tensor(nmr, mv[:, 0:1], rstd, OP.mult)
                nc.vector.tensor_scalar_mul(nmr, nmr, -1.0)
                hs = []
                for sg in range(2):
                    h = temps.tile(
                        [P, NOWN], BF16, tag=f"ln_h{sg}", name=f"h{sg}"
                    )
                    nc.scalar.activation(
                        out=h, in_=halves[sg], func=AF.Identity,
                        bias=nmr, scale=rstd,
                    )
                    hs.append(h)
                return hs

            # ---------- persistent attention/MLP SBUF ----------
            # hT[(j, t2)]: [128, 2, 512] fp8, c-slabs (2j, 2j+1), tok half t2
            hT = {
                (j, t2): persist.tile(
                    [P, 2, NOWN], FP8, tag=f"hT{j}_{t2}", name=f"hT{j}_{t2}"
                )
                for j in range(4)
                for t2 in range(2)
            }
            kT = {
                (ft, t2): persist.tile(
                    [P, NOWN], BF16, tag=f"kT{ft}_{t2}", name=f"kT{ft}_{t2}"
                )
                for ft in range(CT)
                for t2 in range(2)
            }
            qT = [
                persist.tile([P, NOWN], BF16, tag=f"qT{ft}", name=f"qT{ft}")
                for ft in range(CT)
            ]
            # vh: [k-token-part, k-tile, head*80 + (64 dims | ones col)] fp8
            vh = persist.tile(
                [P, TT, HEADS * HW8], FP8, tag="vh", name="vh"
            )
            # oT pairs for proj DoubleRow: [feat-part, slab-pair, tok] fp8
            oT = [
                persist.tile([P, 2, NOWN], FP8, tag=f"oT{j}", name=f"oT{j}")
                for j in range(4)
            ]
            h2T = [
                persist.tile([P, NOWN], BF16, tag=f"h2T{c}", name=f"h2T{c}")
                for c in range(CT)
            ]
            vh4 = vh.rearrange("p t (h w) -> p t h w", h=HEADS)
            nc.vector.memset(vh4[:, :, :, DH : DH + 1], 1.0)

            # ====== Phase 1: per token tile: LN1 -> hT, then V(t).  Tiles
            # 4-7 also interleave K(*, half0) + Q (their hT is complete
            # after tile 3), keeping PE dense while x still streams in. ===
            with (
                tc.tile_pool(name="ln1", bufs=3) as ln1,
                tc.tile_pool(name="tr1", bufs=2, space="PSUM") as tr1,
                tc.tile_pool(name="v_ps", bufs=3, space="PSUM") as v_ps,
            ):

                def kq_gemm(pool, slab, t2, bias, dest):
                    ps = pool.tile([P, NOWN], F32, tag="vps", name="kq")
                    for j in range(4):
                        nc.tensor.matmul(
                            ps,
                            lhsT=slab[:, 2 * j : 2 * j + 2, :],
                            rhs=hT[(j, t2)],
                            start=(j == 0),
                            stop=(j == 3),
                            perf_mode=DR,
                        )
                    nc.vector.tensor_scalar(
                        out=dest, in0=ps, scalar1=RWS, scalar2=bias,
                        op0=OP.mult, op1=OP.add,
                    )

                for t in range(TT):
                    xt = xt_all[t]
                    hs = layernorm_tile(
                        ln1, [xt[:, 0:NOWN], xt[:, NOWN:C]]
                    )
                    t2, tb = t // QT, t % QT
                    for c in range(CT):
                        ps = tr1.tile([P, P], BF16, tag="tr", name="tr")
                        src_h = hs[c // 4][:, (c % 4) * P : (c % 4 + 1) * P]
                        nc.tensor.transpose(ps, src_h, ident)
                        copy_alt(
                            c,
                            hT[(c // 2, t2)][:, c % 2, tb * P : (tb + 1) * P],
                            ps,
                        )
                    # -- V(t): token-major; vh slice = (psum + 16*vb)/16 --
                    for ns in range(2):
                        ps = v_ps.tile([P, NOWN], F32, tag="vps", name="v")
                        for j in range(4):
                            nc.tensor.matmul(
                                ps,
                                lhsT=hT[(j, t2)][:, :, tb * P : (tb + 1) * P],
                                rhs=vslab[ns][:, 2 * j : 2 * j + 2, :],
                                start=(j == 0),
                                stop=False,
                                perf_mode=DR,
                            )
                        # rank-1 bias add: ones(tok) x 16*vb(feat)
                        nc.tensor.matmul(
                            ps,
                            lhsT=ones8,
                            rhs=vb8[:, ns * NOWN : (ns + 1) * NOWN],
                            start=False,
                            stop=True,
                        )
                        out_view = vh4[:, t, ns * 8 : (ns + 1) * 8, :DH]
                        nc.vector.tensor_scalar(
                            out=out_view, in0=ps, scalar1=RWS,
                            scalar2=None, op0=OP.mult,
                        )
                    if t >= QT:
                        for ft in (2 * (t - QT), 2 * (t - QT) + 1):
                            kq_gemm(
                                v_ps, kslab[ft], 0,
                                kbT[:, ft : ft + 1], kT[(ft, 0)],
                            )
                            kq_gemm(
                                v_ps, qslab[ft], 0,
                                qbT[:, ft : ft + 1], qT[ft],
                            )

            # ====== Phase 2: per ft: K(ft, half1), then heads (pipelined
            # by 1 head so the in-order PE queue never waits on exp) ======
            with (
                tc.tile_pool(name="st_ps", bufs=3, space="PSUM") as st_ps,
                tc.tile_pool(name="ot_ps", bufs=2, space="PSUM") as ot_ps,
                tc.tile_pool(name="den_sb", bufs=2) as den_sb,
                tc.tile_pool(name="p_sb", bufs=2) as p_pool,
            ):

                def attn_out(h_idx, pg):
                    """attn@V + softmax-normalize for one head (emitted one
                    head late so the PE never waits on the exp chain)."""
                    ft, hh = h_idx // 2, h_idx % 2
                    prow = hh * DH
                    ot = ot_ps.tile([P, NOWN], F32, tag="ot", name="ot")
                    hc0 = h_idx * HW8
                    for j in range(4):
                        nc.tensor.matmul(
                            ot[: DH + 1, :],
                            lhsT=vh[
                                :, 2 * j : 2 * j + 2, hc0 : hc0 + DH + 1
                            ],
                            rhs=pg[j],
                            start=(j == 0),
                            stop=(j == 3),
                            perf_mode=DR,
                        )
                    # softmax denominator in row DH (vh ones column);
                    # oT = 16 * o / den  (x16 = fp8 range for proj)
                    # rs = 16/den: stage den/16 in SBUF (the custom
                    # DVE reciprocal op cannot read PSUM), then invert
                    dsb = den_sb.tile([1, NOWN], F32, tag="dsb", name="d")
                    nc.vector.tensor_scalar(
                        out=dsb, in0=ot[DH : DH + 1, :], scalar1=RWS,
                        scalar2=None, op0=OP.mult,
                    )
                    rs = den_sb.tile([1, NOWN], F32, tag="rs", name="rs")
                    nc.vector.reciprocal_approx_fast(out=rs, in_=dsb)
                    rsb = den_sb.tile([DH, NOWN], F32, tag="rb", name="rb")
                    nc.gpsimd.partition_broadcast(rsb, rs)
                    nc.vector.tensor_tensor(
                        oT[ft // 2][prow : prow + DH, ft % 2, :],
                        ot[:DH, :],
                        rsb,
                        OP.mult,
                    )

                prev = None
                for ft in range(CT):
                    # K(ft, half1): psum borrowed from the st pool (row 0)
                    # so st+ot fill all 8 banks with 3-deep score buffering
                    psk = st_ps.tile([P, 2, NOWN], F32, tag="st", name="k")
                    for j in range(4):
                        nc.tensor.matmul(
                            psk[:, 0, :],
                            lhsT=kslab[ft][:, 2 * j : 2 * j + 2, :],
                            rhs=hT[(j, 1)],
                            start=(j == 0),
                            stop=(j == 3),
                            perf_mode=DR,
                        )
                    nc.vector.tensor_scalar(
                        out=kT[(ft, 1)],
                        in0=psk[:, 0, :],
                        scalar1=RWS,
                        scalar2=kbT[:, ft : ft + 1],
                        op0=OP.mult,
                        op1=OP.add,
                    )

                    # late weight DMAs (proj needed only after attention)
                    if 1 <= ft <= 4:
                        ns, j2 = (ft - 1) // 2, 2 * ((ft - 1) % 2)
                        for jj in (j2, j2 + 1):
                            nc.gpsimd.dma_start(
                                pslab[ns][:, 2 * jj : 2 * jj + 2, :],
                                io["pw"][ns, jj, :, :, :],
                            )
                    if ft == 4:
                        pb_bc = bcast_const(io["pb"], C)

                    for hh in range(2):
                        h_idx = 2 * ft + hh
                        prow = hh * DH
                        pg = [
                            p_pool.tile(
                                [P, 2, NOWN], FP8, tag=f"p{g}", name="p"
                            )
                            for g in range(4)
                        ]
                        for g in range(4):  # pairs of k-tiles
                            stg = st_ps.tile(
                                [P, 2, NOWN], F32, tag="st", name="st"
                            )
                            for i in range(2):
                                c = 2 * g + i
                                nc.tensor.matmul(
                                    stg[:, i, :],
                                    lhsT=kT[(ft, c // 4)][
                                        prow : prow + DH,
                                        (c % 4) * P : (c % 4 + 1) * P,
                                    ],
                                    rhs=qT[ft][prow : prow + DH, :],
                                    start=True,
                                    stop=True,
                                )
                            nc.scalar.activation(
                                out=pg[g], in_=stg, func=AF.Exp, scale=SCALE
                            )
                        if prev is not None:
                            attn_out(*prev)
                        prev = (h_idx, pg)
                attn_out(*prev)

            _xwa_stack.close()  # frees x_other + k/q/v slabs for FC pools

            # 256*(x_own + pb) precomputed so the proj drain is ONE add:
            # x2 is accumulated at 256x scale (psum already carries the
            # 16*16 weight prescales); LN2 is scale-invariant with eps
            # scaled by 256^2, and FC2's residual applies the 1/256.
            xpb = [
                xwb.tile([P, C], F32, tag=f"xpb{t}", name=f"xpb{t}")
                for t in range(QT)
            ]
            for t in range(QT):
                nc.vector.tensor_add(xpb[t], x_own[t], pb_bc)
                nc.vector.tensor_scalar_mul(xpb[t], xpb[t], WS * WS)

            # ==== Phase 4+5: proj + residual -> x2, LN2 -> h2T (per tq) ==
            with (
                tc.tile_pool(name="proj_ps", bufs=4, space="PSUM") as proj_ps,
                tc.tile_pool(name="tr2", bufs=2, space="PSUM") as tr2,
                tc.tile_pool(name="ln2", bufs=2) as ln2,
            ):
                for tq in range(QT):
                    for ns in range(2):
                        nsl = slice(ns * NOWN, (ns + 1) * NOWN)
                        ps = proj_ps.tile([P, NOWN], F32, tag="pps", name="pp")
                        for j2 in range(4):
                            nc.tensor.matmul(
                                ps,
                                lhsT=oT[j2][:, :, tq * P : (tq + 1) * P],
                                rhs=pslab[ns][:, 2 * j2 : 2 * j2 + 2, :],
                                start=(j2 == 0),
                                stop=(j2 == 3),
                                perf_mode=DR,
                            )
                        # x2_256 = psum + 256*(pb + x_own)
                        nc.vector.tensor_add(
                            x2[(tq, ns)], ps, xpb[tq][:, nsl]
                        )
                    hs = layernorm_tile(
                        ln2, [x2[(tq, 0)], x2[(tq, 1)]], eps_t=eps256
                    )
                    for c in range(CT):
                        ps2 = tr2.tile([P, P], BF16, tag="tr", name="tr")
                        src_h = hs[c // 4][:, (c % 4) * P : (c % 4 + 1) * P]
                        nc.tensor.transpose(ps2, src_h, ident)
                        copy_alt(c, h2T[c][:, tq * P : (tq + 1) * P], ps2)

        # ============ Phase 6+7: FC1 + gelu -> actT (fp8 hc-pairs), =======
        # ============ FC2 fp8 DoubleRow + residual                  =======
        actT = [
            persist.tile(
                [P, 2, NOWN], FP8, tag=f"actT{g}", name=f"actT{g}"
            )
            for g in range(HT // 2)
        ]
        with (
            tc.tile_pool(name="f1c", bufs=4) as f1c,
            tc.tile_pool(name="f1_ps", bufs=4, space="PSUM") as f1_ps,
            tc.tile_pool(name="f2c", bufs=1) as f2c,
            tc.tile_pool(name="f2_ps", bufs=2, space="PSUM") as f2_ps,
            tc.tile_pool(name="out_sb", bufs=2) as out_pool,
        ):
            # fc2 weight groups (fp8, both halves); DMAs are spread through
            # the fc1 loop below so fc1's own slab DMAs stay ahead
            groups = {
                (ns, g): f2c.tile(
                    [P, 4, NOWN], FP8, tag=f"g{ns}_{g}", name=f"fg{ns}{g}"
                )
                for ns in range(2)
                for g in range(8)
            }

            def fetch_group(ns, g):
                nc.gpsimd.dma_start(groups[(ns, g)], io["f2w"][ns, g, :, :, :])

            for hf in range(HT):
                ps = f1_ps.tile([P, NOWN], F32, tag="f1ps", name="f1ps")
                slab = f1c.tile([P, CT, P], BF16, tag="f1w", name="f1slab")
                for q in range(2):  # 2 chunks -> 2 rings per slab
                    eng = nc.sync if q == 0 else nc.gpsimd
                    eng.dma_start(
                        slab[:, 4 * q : 4 * q + 4, :],
                        io["f1w"][hf, q, :, :, :],
                    )
                if hf < 16:  # one fc2 group DMA per fc1 iteration
                    fetch_group(hf % 2, hf // 2)
                elif hf == 16:
                    f2b_bc = bcast_const(io["f2b"], C)
                for c in range(CT):
                    nc.tensor.matmul(
                        ps,
                        lhsT=slab[:, c, :],
                        rhs=h2T[c],
                        start=(c == 0),
                        stop=(c == CT - 1),
                    )
                nc.scalar.activation(
                    out=actT[hf // 2][:, hf % 2, :],
                    in_=ps,
                    func=AF.Gelu,
                    bias=f1bT[:, hf : hf + 1],
                    scale=1.0,
                )

            # rewrite x2 in place to (x + attn_resid + pb + f2b): the fc2
            # drain is then psum/16 + x2.  DVE is idle during fc1.
            for tq in range(QT):
                for ns in range(2):
                    nsl = slice(ns * NOWN, (ns + 1) * NOWN)
                    t2x = x2[(tq, ns)]
                    nc.vector.tensor_scalar_mul(t2x, t2x, 1.0 / (WS * WS))
                    nc.vector.tensor_tensor(
                        t2x, t2x, f2b_bc[:, nsl], OP.add
                    )

            for ns in range(2):
                nsl = slice(ns * NOWN, (ns + 1) * NOWN)
                for tq in range(QT):
                    ps = f2_ps.tile([P, NOWN], F32, tag="f2ps", name="f2ps")
                    for m in range(HT // 2):  # pairs of hidden slabs
                        nc.tensor.matmul(
                            ps,
                            lhsT=actT[m][:, :, tq * P : (tq + 1) * P],
                            rhs=groups[(ns, m // 2)][
                                :, 2 * (m % 2) : 2 * (m % 2) + 2, :
                            ],
                            start=(m == 0),
                            stop=(m == HT // 2 - 1),
                            perf_mode=DR,
                        )
                    ot2 = out_pool.tile([P, NOWN], F32, tag="out_t", name="o")
                    nc.vector.tensor_scalar_mul(ps, ps, RWS)
                    nc.vector.tensor_add(ot2, ps, x2[(tq, ns)])
                    for q in range(4):  # 4 rings so the tail isn't DMA-bound
                        eng = nc.sync if q % 2 == 0 else nc.gpsimd
                        eng.dma_start(
                            out_d[tq, ns, q, :, :],
                            ot2[:, q * P : (q + 1) * P],
                        )


_PROGRAM = None


def _get_program():
    global _PROGRAM
    if _PROGRAM is None:
        _PROGRAM = build_program()
    return _PROGRAM


def build_in_maps(inputs):
    import ml_dtypes

    E4 = ml_dtypes.float8_e4m3

    x = np.asarray(inputs["x"], np.float32)  # [4, 1024, 1024]
    ln1_g = np.asarray(inputs["ln1_g"], np.float64)
    ln1_b = np.asarray(inputs["ln1_b"], np.float64)
    ln2_g = np.asarray(inputs["ln2_g"], np.float64)
    ln2_b = np.asarray(inputs["ln2_b"], np.float64)
    qkv_w = np.asarray(inputs["qkv_w"], np.float64)
    qkv_b = np.asarray(inputs["qkv_b"], np.float64)
    proj_w = np.asarray(inputs["proj_w"], np.float64)
    proj_b = np.asarray(inputs["proj_b"], np.float32)
    fc1_w = np.asarray(inputs["fc1_w"], np.float64)
    fc1_b = np.asarray(inputs["fc1_b"], np.float64)
    fc2_w = np.asarray(inputs["fc2_w"], np.float64)
    fc2_b = np.asarray(inputs["fc2_b"], np.float32)

    # Fold LN affine into the following matmul:
    #   (xhat*g + b) @ W == xhat @ (diag(g) W) + b @ W
    qkv_w_f = ln1_g[:, None] * qkv_w
    qkv_b_f = (qkv_b + ln1_b @ qkv_w).astype(np.float32)
    f1w_f = ln2_g[:, None] * fc1_w
    f1b_f = (fc1_b + ln2_b @ fc1_w).astype(np.float32)

    qw8 = (qkv_w_f[:, :C] * WS).astype(np.float32).astype(E4)
    kw8 = (qkv_w_f[:, C : 2 * C] * WS).astype(np.float32).astype(E4)
    vw8 = (qkv_w_f[:, 2 * C :] * WS).astype(np.float32).astype(E4)
    pw8 = (proj_w * WS).astype(np.float32).astype(E4)
    f1w16 = f1w_f.astype(ml_dtypes.bfloat16)
    f2w8 = (fc2_w * WS).astype(np.float32).astype(E4)

    # DMA-friendly tilings: each dma_start source is one contiguous block
    def vp_tile(w8):  # [C, C] -> [2, 4, P, 2, NOWN]  (ns, j, p, c2, n)
        return np.ascontiguousarray(
            w8.reshape(4, 2, P, 2, NOWN).transpose(3, 0, 2, 1, 4)
        )

    def kq_tile(w8):  # [C, C] -> [CT, P, CT, P]  (ft, p, c, f)
        return np.ascontiguousarray(
            w8.reshape(CT, P, CT, P).transpose(2, 1, 0, 3)
        )

    f1t = np.ascontiguousarray(  # [HT, 2, P, 4, P]  (hf, u, p, c4, f)
        f1w16.reshape(2, 4, P, HT, P).transpose(3, 0, 2, 1, 4)
    )
    f2t = np.ascontiguousarray(  # [2, 8, P, 4, NOWN]  (ns, g, p, o, n)
        f2w8.reshape(8, 4, P, 2, NOWN).transpose(3, 0, 2, 1, 4)
    )

    def tbias(b):  # [n*128] -> [128, n] per-partition layout
        return np.ascontiguousarray(b.reshape(-1, P).T)

    common = dict(
        qw=kq_tile(qw8),
        kw=kq_tile(kw8),
        vw=vp_tile(vw8),
        pw=vp_tile(pw8),
        f1w=f1t,
        f2w=f2t,
        qbt=tbias(qkv_b_f[:C]),
        kbt=tbias(qkv_b_f[C : 2 * C]),
        f1bt=tbias(f1b_f),
        vb=np.ascontiguousarray(qkv_b_f[2 * C :]),
        pb=proj_b,
        f2b=fc2_b,
    )
    in_maps = []
    for core in range(8):
        b, half = core // 2, core % 2
        own = x[b, half * NOWN : (half + 1) * NOWN, :]
        other = x[b, (1 - half) * NOWN : (2 - half) * NOWN, :]
        xp = np.concatenate([own, other], axis=0)
        # [1024, 1024] -> [TT, 4, P, 256]  (t, q, p, n)
        xt = np.ascontiguousarray(
            xp.reshape(TT, P, 4, C // 4).transpose(0, 2, 1, 3)
        )
        in_maps.append({**common, "x": xt})
    return in_maps


def kernel(**inputs):
    in_maps = build_in_maps(inputs)
    nc = _get_program()
    res = run_bass_kernel_spmd(nc, in_maps, core_ids=list(range(8)))
    outs = res.results

    y = np.empty((4, NTOK, C), np.float32)
    for core in range(8):
        b, half = core // 2, core % 2
        # out: [QT, 2, 4, P, 128]  (tq, ns, q, p, n) -> [NOWN, C]
        yc = outs[core]["out"].transpose(0, 3, 1, 2, 4).reshape(NOWN, C)
        y[b, half * NOWN : (half + 1) * NOWN, :] = yc
    return y


if __name__ == "__main__":
    prog = build_program()
    print("program built OK")


# revision 29
# speedup vs baseline: 1.3342x; 1.3342x over previous
"""Trainium2 Bass kernel for a dense transformer block (pre-LN, MHA + MLP).

Full inputs in, full outputs out. Sharding: 8 cores = (batch, seq-half).
Each core computes K/V over its batch element's full 1024 tokens and
Q/attention/MLP over its own 512 tokens (host permutes tokens so the core's
own half is always rows 0..511 — softmax over keys is permutation invariant).
No collectives needed.

Precision strategy:
  - Attention-side GEMMs (K, V, Q, attn@V, proj) run as fp8e4m3 DoubleRow
    matmuls.  Weights are prescaled x16 on the host to sit in e4m3's normal
    range; the 1/16 (or 1/256) correction folds into the psum drain.
  - fc1 stays bf16 (two fp8 MLP layers would blow the 2e-2 error gate);
    fc2 runs fp8 DoubleRow (acts + weights), which measures ~1.7e-2.
  - Scores (64-deep contraction) stay bf16; DoubleRow needs 128-pairs.

Schedule (the big wins over the first working version):
  - V sweep is interleaved into the LN1 loop: V(token tile t) only needs
    tile t's hT columns, so it streams right behind the LN1 transposes
    instead of waiting for all of LN1.
  - The per-head attention loop is software-pipelined by one head:
    attn@V(h-1) is emitted AFTER scores/exp(h), so the in-order PE queue
    never stalls waiting for the Scalar engine's exp — the phase runs at
    the exp roofline (~60us) instead of ping-ponging.
  - fc2 weight DMAs (both halves) are spread through the fc1 loop so fc2
    never waits on HBM; fc1 slabs are split into 2 chunks / 2 rings.
  - x2 is rewritten in place to (x + attn_resid + pb + f2b) during fc1 so
    the fc2 drain is scale + one add.
"""

import sys

sys.path.insert(0, "/opt/trn_rl_repo")

import numpy as np

import concourse.bass as bass
import concourse.bacc as bacc
import concourse.mybir as mybir
import concourse.tile as tile
from concourse.bass_utils import run_bass_kernel_spmd
from concourse.masks import make_identity

P = 128
C = 1024
HEADS = 16
DH = 64
HID = 4096
NTOK = 1024  # tokens per batch element (kv length)
NOWN = 512  # tokens owned by this core (q length)
SCALE = DH ** -0.5
EPS = 1e-5
WS = 16.0  # fp8 weight prescale
RWS = 1.0 / WS

F32 = mybir.dt.float32
BF16 = mybir.dt.bfloat16
FP8 = mybir.dt.float8e4
AF = mybir.ActivationFunctionType
OP = mybir.AluOpType
DR = mybir.MatmulPerfMode.DoubleRow

CT = C // P  # 8 column tiles of the model dim
TT = NTOK // P  # 8 token tiles (kv)
QT = NOWN // P  # 4 token tiles (own)
HT = HID // P  # 32 hidden tiles
HW8 = 80  # per-head column stride in the fused vh tile (64 dims +
# ones col + pad).  The DoubleRow pair dim is the k-tile dim with stride
# HEADS*HW8 = 1280 bytes (multiple of 16 as dual-fp8 ldweights requires).


def build_program():
    nc = bacc.Bacc("TRN2", target_bir_lowering=False)

    # All big operands are pre-tiled on the host into the exact SBUF
    # destination layout, so every dma_start below is ONE contiguous
    # block copy (strided-DRAM reads measured only ~10 GB/s per ring).
    io = {}
    io["x"] = nc.dram_tensor("x", (TT, 4, P, C // 4), F32, kind="ExternalInput")
    io["qw"] = nc.dram_tensor("qw", (CT, P, CT, P), FP8, kind="ExternalInput")
    io["kw"] = nc.dram_tensor("kw", (CT, P, CT, P), FP8, kind="ExternalInput")
    io["vw"] = nc.dram_tensor(
        "vw", (2, 4, P, 2, NOWN), FP8, kind="ExternalInput"
    )
    io["pw"] = nc.dram_tensor(
        "pw", (2, 4, P, 2, NOWN), FP8, kind="ExternalInput"
    )
    io["f1w"] = nc.dram_tensor(
        "f1w", (HT, 2, P, 4, P), BF16, kind="ExternalInput"
    )
    io["f2w"] = nc.dram_tensor(
        "f2w", (2, 8, P, 4, NOWN), FP8, kind="ExternalInput"
    )
    # biases pre-transposed on host into [128, n] per-partition layout
    io["qbt"] = nc.dram_tensor("qbt", (P, CT), F32, kind="ExternalInput")
    io["kbt"] = nc.dram_tensor("kbt", (P, CT), F32, kind="ExternalInput")
    io["f1bt"] = nc.dram_tensor("f1bt", (P, HT), F32, kind="ExternalInput")
    io["vb"] = nc.dram_tensor("vb", (C,), F32, kind="ExternalInput")
    io["pb"] = nc.dram_tensor("pb", (C,), F32, kind="ExternalInput")
    io["f2b"] = nc.dram_tensor("f2b", (C,), F32, kind="ExternalInput")
    io["out"] = nc.dram_tensor(
        "out", (QT, 2, 4, P, P), F32, kind="ExternalOutput"
    )

    with tile.TileContext(nc) as tc:
        _emit(nc, tc, io)
    nc.compile()
    return nc


def _emit(nc, tc, io):
    x_d, out_d = io["x"], io["out"]

    with (
        tc.tile_pool(name="consts", bufs=1) as consts,
        tc.tile_pool(name="persist", bufs=1) as persist,
    ):

        def copy_alt(i, out, in_):
            nc.any.tensor_copy(out=out, in_=in_)

        import contextlib
        _xwa_stack = contextlib.ExitStack()
        with tc.tile_pool(name="xwb", bufs=1) as xwb:
            xwa = _xwa_stack.enter_context(tc.tile_pool(name="xwa", bufs=1))

            # DMA issue costs ~650ns per dma_start on the issuing queue.
            # Only sync + scalar are HWDGE engines (gpsimd is the slow
            # SWDGE path — measured much worse).  x alternates the two so
            # the startup enqueue isn't fully serial; scalar is otherwise
            # idle until LN1 h-writes begin.
            engs = [nc.sync, nc.scalar]
            _ei = [0]

            def dma(out, in_):
                engs[_ei[0] % 2].dma_start(out, in_)
                _ei[0] += 1

            # ---- x tiles first: LN1 is the critical path at startup ----
            xt_all = []
            for t in range(TT):
                pool_t = xwb if t < QT else xwa
                xo = pool_t.tile([P, C], F32, tag=f"xo{t}", name=f"xo{t}")
                for q in range(4):
                    cs = slice(q * 256, (q + 1) * 256)
                    dma(xo[:, cs], x_d[t, q, :, :])
                xt_all.append(xo)
            x_own = xt_all[:QT]

            # vb: only a single-partition row is ever needed (rank-1 trick)
            vbrow = xwa.tile([1, C], F32, tag="vbrow", name="vbrow")
            nc.sync.dma_start(
                vbrow, bass.AP(tensor=io["vb"], offset=0, ap=[[0, 1], [1, C]])
            )
            nc.vector.tensor_scalar_mul(vbrow, vbrow, WS)
            vb8 = xwa.tile([1, C], FP8, tag="vb8", name="vb8")
            nc.vector.tensor_copy(out=vb8, in_=vbrow)
            ones8 = xwa.tile([1, P], FP8, tag="ones8", name="ones8")
            nc.vector.memset(ones8, 1.0)

            # fp8 V weight slabs right behind the early x tiles (V streams
            # inside the LN1 loop)
            vslab = [
                xwa.tile([P, CT, NOWN], FP8, tag=f"vs{ns}", name=f"vs{ns}")
                for ns in range(2)
            ]
            for ns in range(2):
                for j in range(4):
                    dma(
                        vslab[ns][:, 2 * j : 2 * j + 2, :],
                        io["vw"][ns, j, :, :, :],
                    )

            # ---------- constants (bias DMAs are small; issue before the
            # big k/q/proj weight slabs so the ft-loop isn't bias-gated) --
            ident = consts.tile([P, P], BF16)
            with tc.tile_pool(name="ident_tmp", bufs=1) as ident_tmp:
                ident_f32 = ident_tmp.tile([P, P], F32)
                make_identity(nc, ident_f32)
                nc.vector.tensor_copy(out=ident, in_=ident_f32)
            eps_tile = consts.tile([P, 1], F32)
            nc.vector.memset(eps_tile, EPS)
            eps256 = consts.tile([P, 1], F32)
            nc.vector.memset(eps256, EPS * (WS * WS) ** 2)

            qbT = consts.tile([P, CT], F32)
            nc.sync.dma_start(qbT, io["qbt"][:, :])
            kbT = consts.tile([P, CT], F32)
            nc.sync.dma_start(kbT, io["kbt"][:, :])
            f1bT = consts.tile([P, HT], F32)
            nc.sync.dma_start(f1bT, io["f1bt"][:, :])

            kslab = [
                xwa.tile([P, CT, P], FP8, tag=f"ks{ft}", name=f"ks{ft}")
                for ft in range(CT)
            ]
            qslab = [
                xwa.tile([P, CT, P], FP8, tag=f"qs{ft}", name=f"qs{ft}")
                for ft in range(CT)
            ]
            for ft in range(CT):
                dma(kslab[ft], io["kw"][ft, :, :, :])
                dma(qslab[ft], io["qw"][ft, :, :, :])
            # pslab DMAs are issued inside the ft-loop (proj runs late)
            pslab = [
                xwb.tile([P, CT, NOWN], FP8, tag=f"pws{ns}", name=f"pws{ns}")
                for ns in range(2)
            ]

            def bcast_const(src_d, n):
                # split across 4 DMA rings; a single 512KB broadcast DMA
                # would occupy one ring for ~30us
                t = consts.tile([P, n], F32)
                for q in range(4):
                    nq = n // 4
                    src = bass.AP(
                        tensor=src_d, offset=q * nq, ap=[[0, P], [1, nq]]
                    )
                    nc.sync.dma_start(t[:, q * nq : (q + 1) * nq], src)
                return t

            # x2 split into (tq, ns) halves so LN2 stats can start after the
            # first proj drain of each token tile (deps are tile-granular)
            x2 = {
                (t, ns): persist.tile(
                    [P, NOWN], F32, tag=f"x2_{t}_{ns}", name=f"x2_{t}_{ns}"
                )
                for t in range(QT)
                for ns in range(2)
            }

            def layernorm_tile(temps, halves, eps_t=None):
                """halves: two [128, 512] fp32 views/tiles of one token tile.
                Returns two normalized bf16 half tiles [128, 512].

                Stats on DVE, h-write on Act (Identity is in every act table).
                """
                stats = temps.tile([P, 2, 6], F32, tag="ln_stats", name="st")
                for sg in range(2):
                    nc.vector.bn_stats(out=stats[:, sg, :], in_=halves[sg])
                mv = temps.tile([P, 2], F32, tag="ln_mv", name="mv")
                nc.vector.bn_aggr(out=mv[:], in_=stats[:])
                rstd = temps.tile([P, 1], F32, tag="ln_rstd", name="rstd")
                nc.scalar.activation(
                    out=rstd, in_=mv[:, 1:2], func=AF.Sqrt,
                    bias=eps_t if eps_t is not None else eps_tile, scale=1.0,
                )
                nc.vector.reciprocal(out=rstd, in_=rstd)
                nmr = temps.tile([P, 1], F32, tag="ln_nmr", name="nmr")
                nc.vector.tensor_tensor(nmr, mv[:, 0:1], rstd, OP.mult)
                nc.vector.tensor_scalar_mul(nmr, nmr, -1.0)
                hs = []
                for sg in range(2):
                    h = temps.tile(
                        [P, NOWN], BF16, tag=f"ln_h{sg}", name=f"h{sg}"
                    )
                    nc.scalar.activation(
                        out=h, in_=halves[sg], func=AF.Identity,
                        bias=nmr, scale=rstd,
                    )
                    hs.append(h)
                return hs

            # ---------- persistent attention/MLP SBUF ----------
            # hT[(j, t2)]: [128, 2, 512] fp8, c-slabs (2j, 2j+1), tok half t2
            hT = {
                (j, t2): persist.tile(
                    [P, 2, NOWN], FP8, tag=f"hT{j}_{t2}", name=f"hT{j}_{t2}"
                )
                for j in range(4)
                for t2 in range(2)
            }
            kT = {
                (ft, t2): persist.tile(
                    [P, NOWN], BF16, tag=f"kT{ft}_{t2}", name=f"kT{ft}_{t2}"
                )
                for ft in range(CT)
                for t2 in range(2)
            }
            qT = [
                persist.tile([P, NOWN], BF16, tag=f"qT{ft}", name=f"qT{ft}")
                for ft in range(CT)
            ]
            # vh: [k-token-part, k-tile, head*80 + (64 dims | ones col)] fp8
            vh = persist.tile(
                [P, TT, HEADS * HW8], FP8, tag="vh", name="vh"
            )
            # oT pairs for proj DoubleRow: [feat-part, slab-pair, tok] fp8
            oT = [
                persist.tile([P, 2, NOWN], FP8, tag=f"oT{j}", name=f"oT{j}")
                for j in range(4)
            ]
            h2T = [
                persist.tile([P, NOWN], BF16, tag=f"h2T{c}", name=f"h2T{c}")
                for c in range(CT)
            ]
            vh4 = vh.rearrange("p t (h w) -> p t h w", h=HEADS)
            nc.vector.memset(vh4[:, :, :, DH : DH + 1], 1.0)

            # ====== Phase 1: per token tile: LN1 -> hT, then V(t).  Tiles
            # 4-7 also interleave K(*, half0) + Q (their hT is complete
            # after tile 3), keeping PE dense while x still streams in. ===
            with (
                tc.tile_pool(name="ln1", bufs=3) as ln1,
                tc.tile_pool(name="tr1", bufs=2, space="PSUM") as tr1,
                tc.tile_pool(name="v_ps", bufs=3, space="PSUM") as v_ps,
            ):

                def kq_gemm(pool, slab, t2, bias, dest):
                    ps = pool.tile([P, NOWN], F32, tag="vps", name="kq")
                    for j in range(4):
                        nc.tensor.matmul(
                            ps,
                            lhsT=slab[:, 2 * j : 2 * j + 2, :],
                            rhs=hT[(j, t2)],
                            start=(j == 0),
                            stop=(j == 3),
                            perf_mode=DR,
                        )
                    nc.vector.tensor_scalar(
                        out=dest, in0=ps, scalar1=RWS, scalar2=bias,
                        op0=OP.mult, op1=OP.add,
                    )

                for t in range(TT):
                    xt = xt_all[t]
                    hs = layernorm_tile(
                        ln1, [xt[:, 0:NOWN], xt[:, NOWN:C]]
                    )
                    t2, tb = t // QT, t % QT
                    for c in range(CT):
                        ps = tr1.tile([P, P], BF16, tag="tr", name="tr")
                        src_h = hs[c // 4][:, (c % 4) * P : (c % 4 + 1) * P]
                        nc.tensor.transpose(ps, src_h, ident)
                        copy_alt(
                            c,
                            hT[(c // 2, t2)][:, c % 2, tb * P : (tb + 1) * P],
                            ps,
                        )
                    # -- V(t): token-major; vh slice = (psum + 16*vb)/16 --
                    for ns in range(2):
                        ps = v_ps.tile([P, NOWN], F32, tag="vps", name="v")
                        for j in range(4):
                            nc.tensor.matmul(
                                ps,
                                lhsT=hT[(j, t2)][:, :, tb * P : (tb + 1) * P],
                                rhs=vslab[ns][:, 2 * j : 2 * j + 2, :],
                                start=(j == 0),
                                stop=False,
                                perf_mode=DR,
                            )
                        # rank-1 bias add: ones(tok) x 16*vb(feat)
                        nc.tensor.matmul(
                            ps,
                            lhsT=ones8,
                            rhs=vb8[:, ns * NOWN : (ns + 1) * NOWN],
                            start=False,
                            stop=True,
                        )
                        out_view = vh4[:, t, ns * 8 : (ns + 1) * 8, :DH]
                        nc.vector.tensor_scalar(
                            out=out_view, in0=ps, scalar1=RWS,
                            scalar2=None, op0=OP.mult,
                        )
                    if t >= QT:
                        for ft in (2 * (t - QT), 2 * (t - QT) + 1):
                            kq_gemm(
                                v_ps, kslab[ft], 0,
                                kbT[:, ft : ft + 1], kT[(ft, 0)],
                            )
                            kq_gemm(
                                v_ps, qslab[ft], 0,
                                qbT[:, ft : ft + 1], qT[ft],
                            )

            # ====== Phase 2: per ft: K(ft, half1), then heads (pipelined
            # by 1 head so the in-order PE queue never waits on exp) ======
            with (
                tc.tile_pool(name="st_ps", bufs=3, space="PSUM") as st_ps,
                tc.tile_pool(name="ot_ps", bufs=2, space="PSUM") as ot_ps,
                tc.tile_pool(name="den_sb", bufs=2) as den_sb,
                tc.tile_pool(name="p_sb", bufs=2) as p_pool,
            ):

                def attn_out(h_idx, pg):
                    """attn@V + softmax-normalize for one head (emitted one
                    head late so the PE never waits on the exp chain)."""
                    ft, hh = h_idx // 2, h_idx % 2
                    prow = hh * DH
                    ot = ot_ps.tile([P, NOWN], F32, tag="ot", name="ot")
                    hc0 = h_idx * HW8
                    for j in range(4):
                        nc.tensor.matmul(
                            ot[: DH + 1, :],
                            lhsT=vh[
                                :, 2 * j : 2 * j + 2, hc0 : hc0 + DH + 1
                            ],
                            rhs=pg[j],
                            start=(j == 0),
                            stop=(j == 3),
                            perf_mode=DR,
                        )
                    # softmax denominator in row DH (vh ones column);
                    # oT = 16 * o / den  (x16 = fp8 range for proj)
                    # rs = 16/den: stage den/16 in SBUF (the custom
                    # DVE reciprocal op cannot read PSUM), then invert
                    dsb = den_sb.tile([1, NOWN], F32, tag="dsb", name="d")
                    nc.vector.tensor_scalar(
                        out=dsb, in0=ot[DH : DH + 1, :], scalar1=RWS,
                        scalar2=None, op0=OP.mult,
                    )
                    rs = den_sb.tile([1, NOWN], F32, tag="rs", name="rs")
                    nc.vector.reciprocal_approx_fast(out=rs, in_=dsb)
                    rsb = den_sb.tile([DH, NOWN], F32, tag="rb", name="rb")
                    nc.gpsimd.partition_broadcast(rsb, rs)
                    nc.vector.tensor_tensor(
                        oT[ft // 2][prow : prow + DH, ft % 2, :],
                        ot[:DH, :],
                        rsb,
                        OP.mult,
                    )

                prev = None
                for ft in range(CT):
                    # K(ft, half1): psum borrowed from the st pool (row 0)
                    # so st+ot fill all 8 banks with 3-deep score buffering
                    psk = st_ps.tile([P, 2, NOWN], F32, tag="st", name="k")
                    for j in range(4):
                        nc.tensor.matmul(
                            psk[:, 0, :],
                            lhsT=kslab[ft][:, 2 * j : 2 * j + 2, :],
                            rhs=hT[(j, 1)],
                            start=(j == 0),
                            stop=(j == 3),
                            perf_mode=DR,
                        )
                    nc.vector.tensor_scalar(
                        out=kT[(ft, 1)],
                        in0=psk[:, 0, :],
                        scalar1=RWS,
                        scalar2=kbT[:, ft : ft + 1],
                        op0=OP.mult,
                        op1=OP.add,
                    )

                    # late weight DMAs (proj needed only after attention)
                    if 1 <= ft <= 4:
                        ns, j2 = (ft - 1) // 2, 2 * ((ft - 1) % 2)
                        for jj in (j2, j2 + 1):
                            nc.sync.dma_start(
                                pslab[ns][:, 2 * jj : 2 * jj + 2, :],
                                io["pw"][ns, jj, :, :, :],
                            )
                    if ft == 4:
                        pb_bc = bcast_const(io["pb"], C)

                    for hh in range(2):
                        h_idx = 2 * ft + hh
                        prow = hh * DH
                        pg = [
                            p_pool.tile(
                                [P, 2, NOWN], FP8, tag=f"p{g}", name="p"
                            )
                            for g in range(4)
                        ]
                        for g in range(4):  # pairs of k-tiles
                            stg = st_ps.tile(
                                [P, 2, NOWN], F32, tag="st", name="st"
                            )
                            for i in range(2):
                                c = 2 * g + i
                                nc.tensor.matmul(
                                    stg[:, i, :],
                                    lhsT=kT[(ft, c // 4)][
                                        prow : prow + DH,
                                        (c % 4) * P : (c % 4 + 1) * P,
                                    ],
                                    rhs=qT[ft][prow : prow + DH, :],
                                    start=True,
                                    stop=True,
                                )
                            nc.scalar.activation(
                                out=pg[g], in_=stg, func=AF.Exp, scale=SCALE
                            )
                        if prev is not None:
                            attn_out(*prev)
                        prev = (h_idx, pg)
                attn_out(*prev)

            _xwa_stack.close()  # frees x_other + k/q/v slabs for FC pools

            # 256*(x_own + pb) precomputed so the proj drain is ONE add:
            # x2 is accumulated at 256x scale (psum already carries the
            # 16*16 weight prescales); LN2 is scale-invariant with eps
            # scaled by 256^2, and FC2's residual applies the 1/256.
            xpb = [
                xwb.tile([P, C], F32, tag=f"xpb{t}", name=f"xpb{t}")
                for t in range(QT)
            ]
            for t in range(QT):
                nc.vector.tensor_add(xpb[t], x_own[t], pb_bc)
                nc.vector.tensor_scalar_mul(xpb[t], xpb[t], WS * WS)

            # ==== Phase 4+5: proj + residual -> x2, LN2 -> h2T (per tq) ==
            with (
                tc.tile_pool(name="proj_ps", bufs=4, space="PSUM") as proj_ps,
                tc.tile_pool(name="tr2", bufs=2, space="PSUM") as tr2,
                tc.tile_pool(name="ln2", bufs=2) as ln2,
            ):
                for tq in range(QT):
                    for ns in range(2):
                        nsl = slice(ns * NOWN, (ns + 1) * NOWN)
                        ps = proj_ps.tile([P, NOWN], F32, tag="pps", name="pp")
                        for j2 in range(4):
                            nc.tensor.matmul(
                                ps,
                                lhsT=oT[j2][:, :, tq * P : (tq + 1) * P],
                                rhs=pslab[ns][:, 2 * j2 : 2 * j2 + 2, :],
                                start=(j2 == 0),
                                stop=(j2 == 3),
                                perf_mode=DR,
                            )
                        # x2_256 = psum + 256*(pb + x_own)
                        nc.vector.tensor_add(
                            x2[(tq, ns)], ps, xpb[tq][:, nsl]
                        )
                    hs = layernorm_tile(
                        ln2, [x2[(tq, 0)], x2[(tq, 1)]], eps_t=eps256
                    )
                    for c in range(CT):
                        ps2 = tr2.tile([P, P], BF16, tag="tr", name="tr")
                        src_h = hs[c // 4][:, (c % 4) * P : (c % 4 + 1) * P]
                        nc.tensor.transpose(ps2, src_h, ident)
                        copy_alt(c, h2T[c][:, tq * P : (tq + 1) * P], ps2)

        # ============ Phase 6+7: FC1 + gelu -> actT (fp8 hc-pairs), =======
        # ============ FC2 fp8 DoubleRow + residual                  =======
        actT = [
            persist.tile(
                [P, 2, NOWN], FP8, tag=f"actT{g}", name=f"actT{g}"
            )
            for g in range(HT // 2)
        ]
        with (
            tc.tile_pool(name="f1c", bufs=4) as f1c,
            tc.tile_pool(name="f1_ps", bufs=4, space="PSUM") as f1_ps,
            tc.tile_pool(name="f2c", bufs=1) as f2c,
            tc.tile_pool(name="f2_ps", bufs=2, space="PSUM") as f2_ps,
            tc.tile_pool(name="out_sb", bufs=2) as out_pool,
        ):
            # fc2 weight groups (fp8, both halves); DMAs are spread through
            # the fc1 loop below so fc1's own slab DMAs stay ahead
            groups = {
                (ns, g): f2c.tile(
                    [P, 4, NOWN], FP8, tag=f"g{ns}_{g}", name=f"fg{ns}{g}"
                )
                for ns in range(2)
                for g in range(8)
            }

            def fetch_group(ns, g):
                nc.scalar.dma_start(groups[(ns, g)], io["f2w"][ns, g, :, :, :])

            for hf in range(HT):
                ps = f1_ps.tile([P, NOWN], F32, tag="f1ps", name="f1ps")
                slab = f1c.tile([P, CT, P], BF16, tag="f1w", name="f1slab")
                for q in range(2):  # 2 chunks -> 2 rings per slab
                    nc.sync.dma_start(
                        slab[:, 4 * q : 4 * q + 4, :],
                        io["f1w"][hf, q, :, :, :],
                    )
                if hf < 16:  # one fc2 group DMA per fc1 iteration
                    fetch_group(hf % 2, hf // 2)
                elif hf == 16:
                    f2b_bc = bcast_const(io["f2b"], C)
                for c in range(CT):
                    nc.tensor.matmul(
                        ps,
                        lhsT=slab[:, c, :],
                        rhs=h2T[c],
                        start=(c == 0),
                        stop=(c == CT - 1),
                    )
                nc.scalar.activation(
                    out=actT[hf // 2][:, hf % 2, :],
                    in_=ps,
                    func=AF.Gelu,
                    bias=f1bT[:, hf : hf + 1],
                    scale=1.0,
                )

            # rewrite x2 in place to (x + attn_resid + pb + f2b): the fc2
            # drain is then psum/16 + x2.  DVE is idle during fc1.
            for tq in range(QT):
                for ns in range(2):
                    nsl = slice(ns * NOWN, (ns + 1) * NOWN)
                    t2x = x2[(tq, ns)]
                    nc.vector.tensor_scalar_mul(t2x, t2x, 1.0 / (WS * WS))
                    nc.vector.tensor_tensor(
                        t2x, t2x, f2b_bc[:, nsl], OP.add
                    )

            # both ns halves accumulate side by side: consecutive matmuls
            # share the same stationary actT slice (LDWEIGHTS reuse)
            for tq in range(QT):
                pss = [
                    f2_ps.tile([P, NOWN], F32, tag=f"f2ps{ns}", name="f2ps")
                    for ns in range(2)
                ]
                for m in range(HT // 2):  # pairs of hidden slabs
                    for ns in range(2):
                        nc.tensor.matmul(
                            pss[ns],
                            lhsT=actT[m][:, :, tq * P : (tq + 1) * P],
                            rhs=groups[(ns, m // 2)][
                                :, 2 * (m % 2) : 2 * (m % 2) + 2, :
                            ],
                            start=(m == 0),
                            stop=(m == HT // 2 - 1),
                            perf_mode=DR,
                        )
                for ns in range(2):
                    ot2 = out_pool.tile([P, NOWN], F32, tag="out_t", name="o")
                    nc.vector.tensor_scalar_mul(pss[ns], pss[ns], RWS)
                    nc.vector.tensor_add(ot2, pss[ns], x2[(tq, ns)])
                    for q in range(4):  # 4 rings so the tail isn't DMA-bound
                        nc.sync.dma_start(
                            out_d[tq, ns, q, :, :],
                            ot2[:, q * P : (q + 1) * P],
                        )


_PROGRAM = None


def _get_program():
    global _PROGRAM
    if _PROGRAM is None:
        _PROGRAM = build_program()
    return _PROGRAM


def build_in_maps(inputs):
    import ml_dtypes

    E4 = ml_dtypes.float8_e4m3

    x = np.asarray(inputs["x"], np.float32)  # [4, 1024, 1024]
    ln1_g = np.asarray(inputs["ln1_g"], np.float64)
    ln1_b = np.asarray(inputs["ln1_b"], np.float64)
    ln2_g = np.asarray(inputs["ln2_g"], np.float64)
    ln2_b = np.asarray(inputs["ln2_b"], np.float64)
    qkv_w = np.asarray(inputs["qkv_w"], np.float64)
    qkv_b = np.asarray(inputs["qkv_b"], np.float64)
    proj_w = np.asarray(inputs["proj_w"], np.float64)
    proj_b = np.asarray(inputs["proj_b"], np.float32)
    fc1_w = np.asarray(inputs["fc1_w"], np.float64)
    fc1_b = np.asarray(inputs["fc1_b"], np.float64)
    fc2_w = np.asarray(inputs["fc2_w"], np.float64)
    fc2_b = np.asarray(inputs["fc2_b"], np.float32)

    # Fold LN affine into the following matmul:
    #   (xhat*g + b) @ W == xhat @ (diag(g) W) + b @ W
    qkv_w_f = ln1_g[:, None] * qkv_w
    qkv_b_f = (qkv_b + ln1_b @ qkv_w).astype(np.float32)
    f1w_f = ln2_g[:, None] * fc1_w
    f1b_f = (fc1_b + ln2_b @ fc1_w).astype(np.float32)

    qw8 = (qkv_w_f[:, :C] * WS).astype(np.float32).astype(E4)
    kw8 = (qkv_w_f[:, C : 2 * C] * WS).astype(np.float32).astype(E4)
    vw8 = (qkv_w_f[:, 2 * C :] * WS).astype(np.float32).astype(E4)
    pw8 = (proj_w * WS).astype(np.float32).astype(E4)
    f1w16 = f1w_f.astype(ml_dtypes.bfloat16)
    f2w8 = (fc2_w * WS).astype(np.float32).astype(E4)

    # DMA-friendly tilings: each dma_start source is one contiguous block
    def vp_tile(w8):  # [C, C] -> [2, 4, P, 2, NOWN]  (ns, j, p, c2, n)
        return np.ascontiguousarray(
            w8.reshape(4, 2, P, 2, NOWN).transpose(3, 0, 2, 1, 4)
        )

    def kq_tile(w8):  # [C, C] -> [CT, P, CT, P]  (ft, p, c, f)
        return np.ascontiguousarray(
            w8.reshape(CT, P, CT, P).transpose(2, 1, 0, 3)
        )

    f1t = np.ascontiguousarray(  # [HT, 2, P, 4, P]  (hf, u, p, c4, f)
        f1w16.reshape(2, 4, P, HT, P).transpose(3, 0, 2, 1, 4)
    )
    f2t = np.ascontiguousarray(  # [2, 8, P, 4, NOWN]  (ns, g, p, o, n)
        f2w8.reshape(8, 4, P, 2, NOWN).transpose(3, 0, 2, 1, 4)
    )

    def tbias(b):  # [n*128] -> [128, n] per-partition layout
        return np.ascontiguousarray(b.reshape(-1, P).T)

    common = dict(
        qw=kq_tile(qw8),
        kw=kq_tile(kw8),
        vw=vp_tile(vw8),
        pw=vp_tile(pw8),
        f1w=f1t,
        f2w=f2t,
        qbt=tbias(qkv_b_f[:C]),
        kbt=tbias(qkv_b_f[C : 2 * C]),
        f1bt=tbias(f1b_f),
        vb=np.ascontiguousarray(qkv_b_f[2 * C :]),
        pb=proj_b,
        f2b=fc2_b,
    )
    in_maps = []
    for core in range(8):
        b, half = core // 2, core % 2
        own = x[b, half * NOWN : (half + 1) * NOWN, :]
        other = x[b, (1 - half) * NOWN : (2 - half) * NOWN, :]
        xp = np.concatenate([own, other], axis=0)
        # [1024, 1024] -> [TT, 4, P, 256]  (t, q, p, n)
        xt = np.ascontiguousarray(
            xp.reshape(TT, P, 4, C // 4).transpose(0, 2, 1, 3)
        )
        in_maps.append({**common, "x": xt})
    return in_maps


def kernel(**inputs):
    in_maps = build_in_maps(inputs)
    nc = _get_program()
    res = run_bass_kernel_spmd(nc, in_maps, core_ids=list(range(8)))
    outs = res.results

    y = np.empty((4, NTOK, C), np.float32)
    for core in range(8):
        b, half = core // 2, core % 2
        # out: [QT, 2, 4, P, 128]  (tq, ns, q, p, n) -> [NOWN, C]
        yc = outs[core]["out"].transpose(0, 3, 1, 2, 4).reshape(NOWN, C)
        y[b, half * NOWN : (half + 1) * NOWN, :] = yc
    return y


if __name__ == "__main__":
    prog = build_program()
    print("program built OK")
